# revision 33
# baseline (speedup 1.0000x reference)
"""Trainium2 Bass kernel for nn_Encoder_48017734369665 (PointNet-style
segment-reduce encoder).

Network (B=16 clouds, N=131072 points, ragged via npts):
    h  = relu(bn1(W1 @ x))            [128, N]
    f  = bn2(W2 @ h)                  [256, N]
    g  = segment_max(f)               [B, 256]
    h3 = relu(bn3(W3 @ [g[seg]; f]))  [512, N]
    h4 = bn4(W4 @ h3)                 [1024, N]
    out = segment_max(h4)             [B, 1024]

Strategy:
  * Inference-mode batchnorms fold into the affine layers on the host.
  * W3 splits into W3g (acting on the per-cloud constant g) and W3f (acting
    on f); the W3g @ g + b3 term is a per-cloud bias vector c computed
    on-device once per cloud from tiny matmuls.
  * Each core owns 2 whole clouds, paired large-with-small so every core's
    padded point count is equal; the SMALL cloud's tiles come first. Each
    cloud's columns are padded to a multiple of F=512 by replicating one of
    its real points — padding is then provably max-invariant and no masking
    of the point axis is needed.
  * Pass 1 computes per-tile maxes of W2'h (layers 1-2) into Mt and stashes
    f itself (bf16) for pass 2. Because slot 0 is the small cloud, its
    per-cloud max g0 only depends on the first kmax tiles: pass 1 runs
    eagerly for those, combine-A produces the slot-0 bias, and pass 2
    (layers 3-4, per-tile maxes of W4'h3) starts on the first ksafe tiles
    (provably slot-0) while the REMAINING pass-1 pairs ride inside the
    pass-2 tile windows, hidden under the L3/L4 matmul stream. Combine-B
    then finishes the slot-1 bias before pass 2 crosses the boundary. Host
    combines per-tile maxes into the final [B, 1024] (adding b4'), so there
    is no cross-device communication.
  * All matmuls run in bf16 (same 1 cycle/row PE rate as float32r, but the
    weight loads ride the fast-weight-load path and hide under the previous
    matmul's stream; fp32 accumulate in PSUM). rel-err tolerance is 2e-2;
    bf16 lands ~6e-3.
  * L4 is software-pipelined half a tile back so the PE never waits on the
    h3 ACT chain, and each PSUM reuse trails its DVE reduce by 8+ matmuls.
"""

import numpy as np
import ml_dtypes

BF16 = ml_dtypes.bfloat16

EPS = 1e-5
B = 16
N = 131072
F = 512  # points per tile (PSUM-bank limit)
NCORES = 8
NEG = -1.0e30


def _fold_bn(W, b, g, be, m, v):
    """bn(W@x + b) == W' @ x + b' with W' = s*W, b' = s*(b-m)+be, s=g/sqrt(v+eps)."""
    s = g / np.sqrt(v + EPS)
    return (s[:, None] * W).astype(np.float32), (s * (b - m) + be).astype(np.float32)


def _cloud_ranges(npts):
    """Per-cloud [start, end) column ranges exactly as the reference's
    jnp.repeat(..., total_repeat_length=N) maps points to clouds: truncate
    if sum > N, extend the last cloud if sum < N."""
    npts = np.maximum(np.asarray(npts, np.int64), 0)
    ends = np.minimum(np.cumsum(npts), N)
    starts = np.concatenate([[0], ends[:-1]])
    ends = ends.copy()
    ends[-1] = N  # pad semantics: trailing points belong to the last cloud
    return [(int(s), int(e)) for s, e in zip(starts, ends)]


def _plan(npts):
    """Pair clouds 2-per-core (small cloud first) so padded per-core tile
    counts are equal.

    Returns (T, ksafe, kmax, slots): slots[c] = [(cloud_id, col_range,
    n_tiles), ...] with slot 0 the smaller cloud; sum(n_tiles) == T for
    every core. Tiles [0, ksafe) belong to slot 0 on every core; slot 0's
    max only depends on tiles [0, kmax).
    """
    ranges = _cloud_ranges(npts)
    sizes = [e - s for s, e in ranges]
    order = np.argsort(sizes)[::-1]  # big → small
    pairs = [(int(order[i]), int(order[2 * NCORES - 1 - i])) for i in range(NCORES)]
    ktiles = [max(1, -(-sizes[b] // F)) for b in range(2 * NCORES)]
    T = max(ktiles[a] + ktiles[b] for a, b in pairs)
    slots = []
    for a, b in pairs:  # a = big, b = small
        kb = ktiles[b]
        slots.append([(b, ranges[b], kb), (a, ranges[a], T - kb)])
    t0s = [s[0][2] for s in slots]
    return T, min(t0s), max(t0s), slots


def _core_inputs(x, T, core_slots, weights):
    """Build the per-core input dict (xs + masks); weights are shared."""
    xs = np.empty((3, T * F), np.float32)
    mneg = np.full((1, 2 * T), NEG, np.float32)
    m01 = np.zeros((1, 2 * T), np.float32)
    t0 = 0
    for slot, (cid, (s, e), ktiles) in enumerate(core_slots):
        n = e - s
        cols = x[:, s:e] if n > 0 else x[:, :1]  # degenerate empty cloud
        n = max(n, 1)
        pad = ktiles * F - n
        if pad > 0:
            cols = np.concatenate([cols, np.repeat(cols[:, :1], pad, axis=1)], axis=1)
        xs[:, t0 * F : (t0 + ktiles) * F] = cols
        mneg[0, slot * T + t0 : slot * T + t0 + ktiles] = 0.0
        m01[0, slot * T + t0 : slot * T + t0 + ktiles] = 1.0
        t0 += ktiles
    weights = dict(weights)
    rowcat = np.concatenate([weights.pop("rowpre"), mneg], axis=1)
    return dict(
        xs=xs.astype(BF16),
        rowcat=np.ascontiguousarray(rowcat).astype(BF16),
        m01=m01.astype(BF16),
        **weights,
    )


def _build_nc(T, ksafe, kmax):
    """Build + compile the SPMD Bass program."""
    import concourse.mybir as mybir
    import concourse.tile as tile
    from concourse import bacc

    f32 = mybir.dt.float32
    bf16 = mybir.dt.bfloat16
    AF = mybir.ActivationFunctionType
    AX = mybir.AxisListType.X

    nc = bacc.Bacc("TRN2", target_bir_lowering=False, debug=False, num_devices=NCORES)

    xs_d = nc.dram_tensor("xs", [3, T * F], bf16, kind="ExternalInput")
    w1t_d = nc.dram_tensor("w1t", [3, 128], bf16, kind="ExternalInput")
    w2t_d = nc.dram_tensor("w2t", [128, 256], bf16, kind="ExternalInput")
    w3gt_d = nc.dram_tensor("w3gt", [128, 2, 512], bf16, kind="ExternalInput")
    w32t_d = nc.dram_tensor("w32t", [128, 512], bf16, kind="ExternalInput")
    w4t_d = nc.dram_tensor("w4t", [128, 4, 1024], bf16, kind="ExternalInput")
    bc_d = nc.dram_tensor("bcat", [128, 3], f32, kind="ExternalInput")
    rc_d = nc.dram_tensor("rowcat", [1, 640 + 2 * T], bf16, kind="ExternalInput")
    m01_d = nc.dram_tensor("m01", [1, 2 * T], bf16, kind="ExternalInput")
    vt_d = nc.dram_tensor("vt", [128, T, 8], f32, kind="ExternalOutput")

    # phase-A pass-1 covers the tiles slot-0's max can depend on
    P1A = min(T, 2 * ((kmax + 1) // 2))
    pairsA = [tuple(u for u in (2 * p, 2 * p + 1) if u < P1A)
              for p in range((P1A + 1) // 2)]
    pairsL = [tuple(u for u in (P1A + 2 * p, P1A + 2 * p + 1) if u < T)
              for p in range((T - P1A + 1) // 2)]
    assert len(pairsL) <= max(1, ksafe - 2), (T, ksafe, kmax)

    # first x chunk (pair 0) rides the fast HWDGE sync queue; the rest
    # streams on gpsimd in chunks, ahead of consumption
    boundsA = [2]
    while boundsA[-1] < P1A:
        boundsA.append(min(P1A, boundsA[-1] + 7))
    assert len(boundsA) >= 3 or P1A <= 9
    boundsL = [P1A]
    while boundsL[-1] < T:
        boundsL.append(min(T, boundsL[-1] + 9))

    with tile.TileContext(nc) as tc:
        with (
            tc.tile_pool(name="const", bufs=1) as cp,
            tc.tile_pool(name="work", bufs=4) as wp,
            tc.tile_pool(name="h3p", bufs=3) as h3p,
        ):
            xs = cp.tile([3, T * F], bf16)
            w1t = cp.tile([3, 128], bf16)
            w2t = cp.tile([128, 256], bf16)
            w3gt = cp.tile([128, 2, 512], bf16)
            w32t = cp.tile([128, 512], bf16)
            w4t = cp.tile([128, 4, 1024], bf16)
            bc = cp.tile([128, 3], f32)
            rc = cp.tile([1, 640 + 2 * T], bf16)
            m01t = cp.tile([1, 2 * T], bf16)
            b1 = bc[:, 0:1]
            b2 = bc[:, 1:3]
            b3r = rc[:, 0:512]
            ones = rc[:, 512:640]
            mneg = rc[:, 640 : 640 + 2 * T]
            m01 = m01t[:]
            Mt = cp.tile([128, T, 2], f32)
            gk = cp.tile([128, 2, 2], bf16)
            cT0 = cp.tile([1, 512], bf16)
            cT1 = cp.tile([1, 512], bf16)
            Cb = cp.tile([128, 4, T], f32)
            V = cp.tile([128, T, 8], f32)
            # stash of all h tiles: pass 2's fused W3f@W2 layer reads these
            hst = cp.tile([128, T * F], bf16)

            xchunks = list(zip(boundsA, boundsA[1:]))
            nc.sync.dma_start(w1t[:], w1t_d.ap())
            nc.sync.dma_start(xs[:, 0 : 2 * F], xs_d.ap()[:, 0 : 2 * F])
            nc.scalar.dma_start(bc[:], bc_d.ap())
            nc.sync.dma_start(w2t[:], w2t_d.ap())
            a, b_ = xchunks[0]
            nc.sync.dma_start(xs[:, a * F : b_ * F], xs_d.ap()[:, a * F : b_ * F])
            nc.sync.dma_start(w3gt[:], w3gt_d.ap())
            nc.sync.dma_start(w32t[:], w32t_d.ap())
            for a, b_ in xchunks[1:]:
                nc.sync.dma_start(
                    xs[:, a * F : b_ * F], xs_d.ap()[:, a * F : b_ * F]
                )
            for a, b_ in zip(boundsL, boundsL[1:]):
                nc.scalar.dma_start(
                    xs[:, a * F : b_ * F], xs_d.ap()[:, a * F : b_ * F]
                )
            nc.sync.dma_start(rc[:], rc_d.ap())
            nc.sync.dma_start(m01t[:], m01_d.ap())
            nc.sync.dma_start(w4t[:], w4t_d.ap())

            # pre-warm ACT function tables while the DMAs stream in
            warm = wp.tile([128, 1], f32, tag="gtmp")
            nc.vector.memset(warm[:], 0.0)
            nc.scalar.activation(warm[:], warm[:], AF.Relu, bias=warm[:])
            nc.scalar.activation(warm[:], warm[:], AF.Identity, bias=warm[:])

            def combine(sl, tlim, pool, ptag, pbufs):
                """Per-cloud max -> gk[:, :, sl] -> cT{sl} from Mt[:, :tlim]."""
                cTs = cT0 if sl == 0 else cT1
                for m in range(2):
                    pmask = pool.tile([128, tlim], f32, tag=ptag, bufs=pbufs)
                    nc.tensor.matmul(
                        pmask[:], ones, mneg[:, sl * T : sl * T + tlim],
                        start=True, stop=True,
                    )
                    cmb = wp.tile([128, tlim], f32, tag="cmb")
                    nc.vector.tensor_add(cmb[:], Mt[:, :tlim, m], pmask[:])
                    gtmp = wp.tile([128, 1], f32, tag="gtmp")
                    nc.vector.reduce_max(gtmp[:], cmb[:], axis=AX)
                    nc.vector.tensor_add(
                        gk[:, m, sl : sl + 1], gtmp[:], b2[:, m : m + 1]
                    )
                pcT = pool.tile([1, 512], f32, tag=ptag, bufs=pbufs)
                nc.tensor.matmul(
                    pcT[:], gk[:, 0, sl : sl + 1], w3gt[:, 0, :],
                    start=True, stop=False,
                )
                nc.tensor.matmul(
                    pcT[:], gk[:, 1, sl : sl + 1], w3gt[:, 1, :],
                    start=False, stop=False,
                )
                nc.tensor.matmul(pcT[:], ones[:, 0:1], b3r, start=False, stop=True)
                nc.vector.tensor_copy(cTs[:], pcT[:])

            # ---- phase A: pass-1 pairs for tiles [0, P1A) + combine-A ----
            with tc.tile_pool(name="psum1", bufs=2, space="PSUM") as pp1:
                for pr in pairsA:
                    n = len(pr)
                    t0 = pr[0]
                    ph = pp1.tile([128, 2, F], f32, tag="p1h", bufs=2,
                                  name=f"ph_{t0}")
                    for i, u in enumerate(pr):
                        nc.tensor.matmul(
                            ph[:, i, :], w1t[:], xs[:, u * F : (u + 1) * F],
                            start=True, stop=True,
                        )
                    nc.scalar.activation(
                        hst[:, t0 * F : (t0 + n) * F], ph[:, :n, :],
                        AF.Relu, bias=b1,
                    )
                    for m in range(2):
                        pf = pp1.tile([128, 2, F], f32, tag="p1f", bufs=2,
                                      name=f"pf_{t0}_{m}")
                        for i, u in enumerate(pr):
                            nc.tensor.matmul(
                                pf[:, i, :], w2t[:, m * 128 : (m + 1) * 128],
                                hst[:, u * F : (u + 1) * F],
                                start=True, stop=True,
                            )
                        nc.vector.reduce_max(
                            Mt[:, t0 : t0 + n, m : m + 1], pf[:, :n, :], axis=AX
                        )
                combine(0, P1A, pp1, "p1h", 2)
                # slot-0 bias columns for the always-slot-0 tiles [0, ksafe)
                for mo in range(4):
                    pC = pp1.tile([128, ksafe], f32, tag="p1h", bufs=2)
                    nc.tensor.matmul(
                        pC[:], cT0[:, mo * 128 : (mo + 1) * 128],
                        m01[:, 0:ksafe], start=True, stop=True,
                    )
                    nc.vector.tensor_copy(Cb[:, mo, :ksafe], pC[:])

            # ---- pass 2 (+ remaining pass-1 pairs inside early windows) --
            with tc.tile_pool(name="psum2", bufs=2, space="PSUM") as pp2:

                def l4_pair(t, h3, pair):
                    p4 = pp2.tile([128, 2, F], f32, tag="p4", bufs=3)
                    for sub in range(2):
                        mo = 2 * pair + sub
                        for k in range(4):
                            nc.tensor.matmul(
                                p4[:, sub, :],
                                w4t[:, k, mo * 128 : (mo + 1) * 128],
                                h3[:, k, :], start=(k == 0), stop=(k == 3),
                            )
                    nc.vector.reduce_max(
                        V[:, t, 2 * pair : 2 * pair + 2], p4[:], axis=AX
                    )

                def flush_v(upto):  # tiles [0, upto) fully reduced
                    if upto and (upto % 8 == 0 or upto == T):
                        a = upto - 8 if upto % 8 == 0 else (upto // 8) * 8
                        nc.sync.dma_start(vt_d.ap()[:, a:upto, :], V[:, a:upto, :])

                def combine_b():
                    combine(1, T, pp2, "p3", 2)
                    for mo in range(4):
                        pC = pp2.tile([128, T - ksafe], f32, tag="p3", bufs=2)
                        nc.tensor.matmul(
                            pC[:], cT0[:, mo * 128 : (mo + 1) * 128],
                            m01[:, ksafe:T], start=True, stop=False,
                        )
                        nc.tensor.matmul(
                            pC[:], cT1[:, mo * 128 : (mo + 1) * 128],
                            m01[:, T + ksafe : 2 * T], start=False, stop=True,
                        )
                        nc.vector.tensor_copy(Cb[:, mo, ksafe:], pC[:])

                def l3_mo(t, h3, mo):
                    p3 = pp2.tile([128, F], f32, tag="p3", bufs=2)
                    nc.tensor.matmul(
                        p3[:], w32t[:, mo * 128 : (mo + 1) * 128],
                        hst[:, t * F : (t + 1) * F], start=True, stop=True,
                    )
                    nc.scalar.activation(
                        h3[:, mo, :], p3[:], AF.Relu, bias=Cb[:, mo, t : t + 1]
                    )

                h3_prev = None
                for t in range(T):
                    ins = pairsL[t] if t < len(pairsL) else None
                    h3 = h3p.tile([128, 4, F], bf16, tag="h3")

                    if ins is None:
                        l3_mo(t, h3, 0)
                        l3_mo(t, h3, 1)
                        if h3_prev is not None:
                            l4_pair(t - 1, h3_prev, 2)
                        l3_mo(t, h3, 2)
                        l3_mo(t, h3, 3)
                        if h3_prev is not None:
                            l4_pair(t - 1, h3_prev, 3)
                            flush_v(t)
                        l4_pair(t, h3, 0)
                        if t < T - 1:
                            l4_pair(t, h3, 1)
                        else:
                            for pr_ in (1, 2, 3):
                                l4_pair(t, h3, pr_)
                    else:
                        # window with an embedded pass-1 pair: its psum
                        # rounds share the p4 ring, ordered so every ring
                        # reuse trails the previous occupant's drain by 8+
                        # matmuls and the relu lands early in the ACT queue
                        n = len(ins)
                        ti = ins[0]
                        ph = pp2.tile([128, 2, F], f32, tag="p4", bufs=3,
                                      name=f"ph_{ti}")
                        for i, u in enumerate(ins):
                            nc.tensor.matmul(
                                ph[:, i, :], w1t[:], xs[:, u * F : (u + 1) * F],
                                start=True, stop=True,
                            )
                        l3_mo(t, h3, 0)
                        l3_mo(t, h3, 1)
                        nc.scalar.activation(
                            hst[:, ti * F : (ti + n) * F], ph[:, :n, :],
                            AF.Relu, bias=b1,
                        )
                        l3_mo(t, h3, 2)
                        l3_mo(t, h3, 3)
                        if h3_prev is not None:
                            l4_pair(t - 1, h3_prev, 2)
                        for m in range(2):
                            pf = pp2.tile([128, 2, F], f32, tag="p4", bufs=3,
                                          name=f"pf_{ti}_{m}")
                            for i, u in enumerate(ins):
                                nc.tensor.matmul(
                                    pf[:, i, :],
                                    w2t[:, m * 128 : (m + 1) * 128],
                                    hst[:, u * F : (u + 1) * F],
                                    start=True, stop=True,
                                )
                            nc.vector.reduce_max(
                                Mt[:, ti : ti + n, m : m + 1], pf[:, :n, :],
                                axis=AX,
                            )
                            if m == 0:
                                l4_pair(t, h3, 0)
                        if h3_prev is not None:
                            l4_pair(t - 1, h3_prev, 3)
                            flush_v(t)
                        l4_pair(t, h3, 1)
                        if t == len(pairsL) - 1:
                            combine_b()
                    h3_prev = h3
                flush_v(T)

    nc.compile()
    return nc


def _prep(x, npts, W1, b1, g1, be1, m1, v1, W2, b2, g2, be2, m2, v2,
          W3, b3, g3, be3, m3, v3, W4, b4, g4, be4, m4, v4):
    """Host-side preprocessing shared by kernel() and the test harness."""
    W1f, b1f = _fold_bn(W1, b1, g1, be1, m1, v1)
    W2f, b2f = _fold_bn(W2, b2, g2, be2, m2, v2)
    W3f_, b3f = _fold_bn(W3, b3, g3, be3, m3, v3)
    W4f, b4f = _fold_bn(W4, b4, g4, be4, m4, v4)

    weights = dict(
        w1t=np.ascontiguousarray(W1f.T).astype(BF16),
        w2t=np.ascontiguousarray(W2f.T).astype(BF16),
        w3gt=np.ascontiguousarray(
            W3f_[:, :256].T.reshape(2, 128, 512).transpose(1, 0, 2)
        ).astype(BF16),
        w32t=np.ascontiguousarray(
            (W3f_[:, 256:] @ W2f).T
        ).astype(BF16),
        w4t=np.ascontiguousarray(
            W4f.T.reshape(4, 128, 1024).transpose(1, 0, 2)
        ).astype(BF16),
        bcat=np.ascontiguousarray(
            np.concatenate([b1f[:, None], b2f.reshape(2, 128).T], axis=1)
        ),
        rowpre=np.concatenate(
            [(b3f + W3f_[:, 256:] @ b2f)[None, :],
             np.ones((1, 128), np.float32)], axis=1
        ),
    )

    T, ksafe, kmax, slots = _plan(npts)
    x = np.asarray(x, np.float32)
    in_maps = [_core_inputs(x, T, core_slots, weights) for core_slots in slots]
    return (T, ksafe, kmax), slots, in_maps, b4f


def _gather(results, slots, b4f):
    """Combine per-core per-tile maxes into the [B, 1024] output."""
    out = np.empty((B, 1024), np.float32)
    for c, core_slots in enumerate(slots):
        vt = results[c]["vt"]  # [128, T, 8]; channel = mo*128 + partition
        Tc = vt.shape[1]
        chan = vt.transpose(2, 0, 1).reshape(1024, Tc)
        t0 = 0
        for cid, _rng, ktiles in core_slots:
            out[cid] = chan[:, t0 : t0 + ktiles].max(axis=1) + b4f
            t0 += ktiles
    return out


def kernel(**inputs):
    from concourse.bass_utils import run_bass_kernel_spmd

    # force host numpy: jax arrays would route host math through the (axon)
    # device backend
    inputs = {k: np.asarray(v) for k, v in inputs.items()}
    plan, slots, in_maps, b4f = _prep(**inputs)
    nc = _build_nc(*plan)
    res = run_bass_kernel_spmd(nc, in_maps, core_ids=list(range(NCORES)))
    return _gather(res.results, slots, b4f)
